# revision 6
# baseline (speedup 1.0000x reference)
"""BoundaryEnhancedLoss on 8 TRN2 NeuronCores — data-parallel over batch.

v2: bf16 pred (halves HBM + d runs in DVE 2x mode), th=2t-1 host layout
(kills the ht2 pass; conv pads memset to -1, H-edge fixed by tiny corr
matmuls), bnd via ACT Square(s') + DVE tensor_scalar is_le (drops the
1x STT + Relu pair), dice reformulated so only sums
S1, S2'=sum th*bnd, PB=sum pt*bnd, PTB'=sum pt*th*bnd are needed:
  S2 = (S2'+S1)/2, PTB = (PTB'+PB)/2, inter = PTB,
  union = S1 - PB + 2*PTB.
CE/focal: pt = sigmoid(th*d), lnp = ln(pt) (accum L), sq = (pt-1)^2,
F' = sum sq*lnp; ce = -L/N, focal = -0.25*F'/N.

Layout: partition p = 32*img + q; h = 128r + 32c + q (4 chunks, CB=4).
Per-core stats [128, 8*4]; host reduces partition groups.
"""
import numpy as np
import ml_dtypes
from contextlib import ExitStack

import concourse.bass as bass
import concourse.tile as tile
from concourse import bacc, mybir
from concourse.bass_utils import run_bass_kernel_spmd
from concourse.tile_rust import add_dep_helper

BF16 = mybir.dt.bfloat16
F32 = mybir.dt.float32
Alu = mybir.AluOpType
Act = mybir.ActivationFunctionType

NCORES = 8
BPC = 4          # images per core
H = W = 512
P = 128
Q = 32           # rows per partition-group strip
CB = 4           # h-blocks (free dim) per chunk
NCHUNK = 4       # chunks: h = 128r + 32c + q
NPIX = 32 * H * W
NST = 8          # stat cols per chunk: S1,S2p,PB,PTBp,L,Fp,(spare)
STW = NCHUNK * NST

USE_TTR = False  # tensor_tensor_reduce for products (unverified mode)


def _band_consts():
    # Block-diagonal 32-bands over q within each 32-partition image group.
    bmain = np.zeros((P, P), dtype=np.float32)
    btop = np.zeros((P, P), dtype=np.float32)   # from block c-1 (q=30,31)
    bbot = np.zeros((P, P), dtype=np.float32)   # from block c+1 (q=0,1)
    for g in range(BPC):
        o = g * Q
        for k in range(Q):
            for m in range(max(0, k - 2), min(Q, k + 3)):
                bmain[o + k, o + m] = 1.0
        btop[o + 30, o + 0] = 1.0
        btop[o + 31, o + 0] = btop[o + 31, o + 1] = 1.0
        bbot[o + 0, o + 30] = bbot[o + 0, o + 31] = 1.0
        bbot[o + 1, o + 31] = 1.0
    # H-edge correction for th=+-1 conv: rows 0,1 miss 2,1 pad rows (each
    # contributing -5 after the W-conv); same for rows 510,511.
    ec0 = np.zeros((1, P), dtype=np.float32)   # chunk 0, block c=0
    ec3 = np.zeros((1, P), dtype=np.float32)   # chunk 3, block c=3
    for g in range(BPC):
        o = g * Q
        ec0[0, o + 0] = -10.0
        ec0[0, o + 1] = -5.0
        ec3[0, o + 30] = -5.0
        ec3[0, o + 31] = -10.0
    bf = ml_dtypes.bfloat16
    return (bmain.astype(bf), btop.astype(bf), bbot.astype(bf),
            ec0.astype(bf), ec3.astype(bf))


def build_nc():
    nc = bacc.Bacc("TRN2", target_bir_lowering=False, debug=False,
                   num_devices=NCORES)
    # host pre-arranged: [ch, r, 32*img+q, c, w] / [r, 32*img+q, c, w]
    pred = nc.dram_tensor("pred", [2, NCHUNK, P, CB, W], BF16,
                          kind="ExternalInput")
    tgt = nc.dram_tensor("tgt", [NCHUNK, P, CB, W], BF16,
                         kind="ExternalInput")
    bmain = nc.dram_tensor("bmain", [P, P], BF16, kind="ExternalInput")
    btop = nc.dram_tensor("btop", [P, P], BF16, kind="ExternalInput")
    bbot = nc.dram_tensor("bbot", [P, P], BF16, kind="ExternalInput")
    ec0 = nc.dram_tensor("ec0", [1, P], BF16, kind="ExternalInput")
    ec3 = nc.dram_tensor("ec3", [1, P], BF16, kind="ExternalInput")
    stats = nc.dram_tensor("stats", [P, STW], F32, kind="ExternalOutput")

    with tile.TileContext(nc) as tc, ExitStack() as ctx:
        persist = ctx.enter_context(tc.tile_pool(name="persist", bufs=1))
        work = ctx.enter_context(tc.tile_pool(name="work", bufs=2))
        psum = ctx.enter_context(tc.tile_pool(name="psum", bufs=2, space="PSUM"))

        bias_m1 = persist.tile([P, 1], F32, tag="bias_m1")
        nc.gpsimd.memset(bias_m1[:], -1.0)
        bmain_t = persist.tile([P, P], BF16, tag="bmain")
        btop_t = persist.tile([P, P], BF16, tag="btop")
        bbot_t = persist.tile([P, P], BF16, tag="bbot")
        ec0_t = persist.tile([1, P], BF16, tag="ec0")
        ec3_t = persist.tile([1, P], BF16, tag="ec3")
        ones_t = persist.tile([1, W], BF16, tag="ones")
        onef = persist.tile([P, CB, W], BF16, tag="onef")
        nc.gpsimd.memset(onef[:], 1.0)
        nc.sync.dma_start(bmain_t[:], bmain[:])
        nc.sync.dma_start(btop_t[:], btop[:])
        nc.sync.dma_start(bbot_t[:], bbot[:])
        nc.sync.dma_start(ec0_t[:], ec0[:])
        nc.sync.dma_start(ec3_t[:], ec3[:])
        nc.gpsimd.memset(ones_t[:], 1.0)

        t_tiles, c_tiles, pt_tiles, st_tiles, d_tiles = [], [], [], [], []
        for r in range(NCHUNK):
            t_tiles.append(persist.tile([P, CB, W + 4], BF16,
                                        tag=f"t{r}", name=f"t{r}"))
            c_tiles.append(persist.tile([P, CB, W], BF16,
                                        tag=f"c{r}", name=f"c{r}"))  # 4-tap
            pt_tiles.append(persist.tile([P, CB, W], BF16,
                                         tag=f"pt{r}", name=f"pt{r}"))
            sts = [persist.tile([P, 1], F32, tag=f"st{r}_{j}",
                                name=f"st{r}_{j}") for j in range(6)]
            for t_ in sts:
                nc.scalar.memzero(t_[:])
            st_tiles.append(sts)

        # ---- Phase 1 (per r): th load + W-conv + pred load + sigmoid chain.
        sig_insts = []
        for r in range(NCHUNK):
            tr, cr, ptr = t_tiles[r], c_tiles[r], pt_tiles[r]
            nc.gpsimd.memset(tr[:, :, 0:2], -1.0)
            nc.gpsimd.memset(tr[:, :, W + 2:W + 4], -1.0)
            nc.sync.dma_start(tr[:, :, 2:W + 2], tgt[r])
            a = work.tile([P, CB, W + 3], BF16, tag="wca")
            nc.gpsimd.tensor_tensor(a[:], tr[:, :, 0:W + 3], tr[:, :, 1:W + 4],
                                    op=Alu.add)
            nc.gpsimd.tensor_tensor(cr[:], a[:, :, 0:W], a[:, :, 2:W + 2],
                                    op=Alu.add)

            p0 = work.tile([P, CB, W], BF16, tag="p0")
            p1 = work.tile([P, CB, W], BF16, tag="p1")
            nc.sync.dma_start(p0[:], pred[0, r])
            nc.sync.dma_start(p1[:], pred[1, r])
            d = work.tile([P, CB, W], BF16, tag="d")
            nc.vector.tensor_tensor(d[:], p1[:], p0[:], op=Alu.subtract)
            hs = work.tile([P, CB, W], BF16, tag="hs")
            nc.vector.tensor_tensor(hs[:], tr[:, :, 2:W + 2], d[:],
                                    op=Alu.mult)
            sig_insts.append(nc.scalar.activation(ptr[:], hs[:], Act.Sigmoid))

        # ---- Phase 2 (per r): band matmuls -> s' = 2s-25, bnd, products ----
        for r in range(NCHUNK):
            tr, cr, ptr, st = t_tiles[r], c_tiles[r], pt_tiles[r], st_tiles[r]
            s = psum.tile([P, CB, W], F32, tag="s")
            for c in range(CB):
                pairs = [(bmain_t, c_tiles[r], t_tiles[r], c)]
                if c > 0:
                    pairs.append((btop_t, c_tiles[r], t_tiles[r], c - 1))
                elif r > 0:
                    pairs.append((btop_t, c_tiles[r - 1], t_tiles[r - 1], CB - 1))
                if c < CB - 1:
                    pairs.append((bbot_t, c_tiles[r], t_tiles[r], c + 1))
                elif r < NCHUNK - 1:
                    pairs.append((bbot_t, c_tiles[r + 1], t_tiles[r + 1], 0))
                corr = None
                if r == 0 and c == 0:
                    corr = ec0_t
                elif r == NCHUNK - 1 and c == CB - 1:
                    corr = ec3_t
                n2 = 2 * len(pairs) + (1 if corr is not None else 0)
                k = 0
                for lhsT, b2t, tt_, cb in pairs:
                    nc.tensor.matmul(s[:, c, :], lhsT[:], b2t[:, cb, :],
                                     start=(k == 0), stop=(k == n2 - 1))
                    k += 1
                    nc.tensor.matmul(s[:, c, :], lhsT[:],
                                     tt_[:, cb, 4:W + 4],
                                     start=False, stop=(k == n2 - 1))
                    k += 1
                if corr is not None:
                    nc.tensor.matmul(s[:, c, :], corr[:], ones_t[:],
                                     start=False, stop=True)
                    k += 1
            # bnd = (s'^2 <= 576): s' odd in [-25,25]; |s'|<=23 <-> boundary
            q2 = work.tile([P, CB, W], BF16, tag="q2")
            nc.scalar.activation(q2[:], s[:], Act.Square)
            bnd = work.tile([P, CB, W], BF16, tag="bnd")
            nc.vector.scalar_tensor_tensor(
                bnd[:], q2[:], 576.0, onef[:], op0=Alu.is_le, op1=Alu.mult,
                accum_out=st[0][:])
            th_ap = tr[:, :, 2:W + 2]
            if USE_TTR:
                tb = work.tile([P, CB, W], BF16, tag="tb")
                nc.vector.tensor_tensor_reduce(
                    tb[:], th_ap, bnd[:], 1.0, 0.0, op0=Alu.mult, op1=Alu.add,
                    accum_out=st[1][:])
                pb = work.tile([P, CB, W], BF16, tag="pb")
                nc.vector.tensor_tensor_reduce(
                    pb[:], ptr[:], bnd[:], 1.0, 0.0, op0=Alu.mult, op1=Alu.add,
                    accum_out=st[2][:])
                ptb = work.tile([P, CB, W], BF16, tag="ptb")
                nc.vector.tensor_tensor_reduce(
                    ptb[:], ptr[:], tb[:], 1.0, 0.0, op0=Alu.mult, op1=Alu.add,
                    accum_out=st[3][:])
            else:
                tb = work.tile([P, CB, W], BF16, tag="tb")
                nc.vector.scalar_tensor_tensor(
                    tb[:], th_ap, 1.0, bnd[:], op0=Alu.mult, op1=Alu.mult,
                    accum_out=st[1][:])
                pb = work.tile([P, CB, W], BF16, tag="pb")
                nc.vector.scalar_tensor_tensor(
                    pb[:], ptr[:], 1.0, bnd[:], op0=Alu.mult, op1=Alu.mult,
                    accum_out=st[2][:])
                ptb = work.tile([P, CB, W], BF16, tag="ptb")
                nc.vector.scalar_tensor_tensor(
                    ptb[:], ptr[:], 1.0, tb[:], op0=Alu.mult, op1=Alu.mult,
                    accum_out=st[3][:])

        # ---- Phase 3 (per r): ln(pt) + focal ----
        for r in range(NCHUNK):
            ptr, st = pt_tiles[r], st_tiles[r]
            lnp = work.tile([P, CB, W], BF16, tag="lnp")
            li = nc.scalar.activation(lnp[:], ptr[:], Act.Ln,
                                      accum_out=st[4][:])
            add_dep_helper(li.ins, sig_insts[-1].ins, sync=False,
                           reason="group ln-set ops after sigmoid-set ops")
            sq = work.tile([P, CB, W], BF16, tag="sq")
            nc.scalar.activation(sq[:], ptr[:], Act.Square, bias=bias_m1[:])
            fo = work.tile([P, CB, W], BF16, tag="fo")
            if USE_TTR:
                nc.vector.tensor_tensor_reduce(
                    fo[:], sq[:], lnp[:], 1.0, 0.0, op0=Alu.mult, op1=Alu.add,
                    accum_out=st[5][:])
            else:
                nc.vector.scalar_tensor_tensor(
                    fo[:], sq[:], 1.0, lnp[:], op0=Alu.mult, op1=Alu.mult,
                    accum_out=st[5][:])

        for r in range(NCHUNK):
            for j in range(6):
                nc.sync.dma_start(stats[:, r * NST + j:r * NST + j + 1],
                                  st_tiles[r][j][:])

    nc.compile()
    return nc


_NC = None


def _get_nc():
    global _NC
    if _NC is None:
        _NC = build_nc()
    return _NC


def _host_combine(stats_all, sum_t=None):
    """stats_all: 8x [128, 32] f32 -> final loss (np.float32)."""
    S1 = np.zeros(32, np.float64)
    S2p = np.zeros(32, np.float64)
    PB = np.zeros(32, np.float64)
    PTBp = np.zeros(32, np.float64)
    L = 0.0
    F = 0.0
    for core, stm in enumerate(stats_all):
        g = stm.astype(np.float64).reshape(BPC, Q, NCHUNK, NST).sum(axis=(1, 2))
        for i in range(BPC):
            gi = core * BPC + i
            S1[gi] += g[i, 0]
            S2p[gi] += g[i, 1]
            PB[gi] += g[i, 2]
            PTBp[gi] += g[i, 3]
        L += g[:, 4].sum()
        F += g[:, 5].sum()
    ce_loss = (-L) / NPIX
    focal = 0.25 * (-F) / NPIX
    PTB = (PTBp + PB) / 2.0
    inter = PTB
    union = S1 - PB + 2.0 * PTB
    dice = 2.0 * inter / (union + 1e-8)
    bdice = 1.0 - dice.mean()
    return np.float32(ce_loss + focal + bdice)


def run_cores(pred, target, trace=False):
    nc = _get_nc()
    bmain, btop, bbot, ec0, ec3 = _band_consts()
    tgt_f = target.astype(np.float32)
    sum_t = tgt_f.astype(np.float64).sum(axis=(1, 2))
    pred = np.asarray(pred, dtype=np.float32)
    in_maps = []
    for core in range(NCORES):
        sl = slice(core * BPC, (core + 1) * BPC)
        # [b, ch, 128r+32c+q, w] -> [ch, r, 32b+q, c, w]
        pl = (pred[sl].reshape(BPC, 2, NCHUNK, CB, Q, W)
              .transpose(1, 2, 0, 4, 3, 5).reshape(2, NCHUNK, P, CB, W)
              .astype(ml_dtypes.bfloat16))
        tl = ((2.0 * tgt_f[sl] - 1.0).reshape(BPC, NCHUNK, CB, Q, W)
              .transpose(1, 0, 3, 2, 4).reshape(NCHUNK, P, CB, W)
              .astype(ml_dtypes.bfloat16))
        in_maps.append({
            "pred": np.ascontiguousarray(pl),
            "tgt": np.ascontiguousarray(tl),
            "bmain": bmain,
            "btop": btop,
            "bbot": bbot,
            "ec0": ec0,
            "ec3": ec3,
        })
    res = run_bass_kernel_spmd(nc, in_maps, list(range(NCORES)), trace=trace)
    stats_all = [res.results[c]["stats"] for c in range(NCORES)]
    return stats_all, sum_t, res.exec_time_ns


def kernel(pred, target):
    stats_all, sum_t, _ = run_cores(pred, target, trace=False)
    return _host_combine(stats_all, sum_t)
